# revision 1
# baseline (speedup 1.0000x reference)
"""CAAN attention kernel for 8 Trainium2 NeuronCores.

Problem: B=8, N=2048, D=256 single-head attention with a rank-1 output head:
    q = x @ Wq.T + bq ; k = x @ Wk.T + bk ; v = x @ Wv.T + bv
    beta = softmax(q @ k.T / sqrt(D))
    scores = (beta @ v) @ Ww.T + bw          -> [B, N]

Sharding: data-parallel over batch, one batch element per core (SPMD with
per-core input maps; no collectives needed).

Per-core algebra (exact, up to fp reassociation):
  S*sqrt(D) = x A x^T + broadcast(g . x_m),  A = Wq^T Wk, g = Wk^T bq
  (the q.bk and bq.bk terms are constant per softmax row and drop out)
  scores[n] = sum_m P[n,m] (x_m . h) + (bv.Ww + bw),    h = Wv^T Ww^T
  (uses sum_m P = 1; the whole V projection collapses to a vector h)

Device pipeline (S layout, queries n on partitions, keys m on the free axis),
bf16 matmuls (exact fp32 accumulation; only input rounding):
  xT[c, m]   PE transposes of x (bf16, 1 cyc/row)
  QT[c, n] = (sum_d A[d, c] xT[d, n] + g[c]) / sqrt(D)
  wb[p, m] = w[m] = sum_c h[c] xT[c, m]   (broadcast via all-equal-columns
             h_mat lhsT, so every partition gets w)
  loop over 16 n-chunks:
    S    = QT_chunk^T @ xT      [128 x 2048] on PE (8 matmuls)
    E    = exp(S) -> bf16, denominator = ACT accum_out   (ScalarE)
    numerator via scalar_tensor_tensor(E * wb, accum_out) (VectorE)
  scores = numer/denom, PE-transposed to [16, 128] and DMA'd out.
Host epilogue: add (bv.Ww + bw), un-permute tokens (token m lives at flat
position (m % 16) * 128 + m // 16 from the DMA-friendly x layout).

Known-good at ~77us HW exec, rel err ~2.8e-3 vs the fp32 reference
(bf16 input rounding; the algebra itself is exact).
"""

import numpy as np

N = 2048
D = 256
NT = N // 128  # 16 m/n chunks
B = 8
SCALE = 1.0 / 16.0  # 1/sqrt(D)

_CACHE = {}


def _build_nc():
    import concourse.bass as bass  # noqa: F401
    import concourse.tile as tile
    from concourse import bacc, mybir
    from concourse.masks import make_identity

    f32 = mybir.dt.float32
    bf16 = mybir.dt.bfloat16

    nc = bacc.Bacc("TRN2", target_bir_lowering=False, debug=False, num_devices=B)

    x_t = nc.dram_tensor("x", [N, D], f32, kind="ExternalInput")
    wq_t = nc.dram_tensor("Wq", [D, D], f32, kind="ExternalInput")
    wk_t = nc.dram_tensor("Wk", [D, D], f32, kind="ExternalInput")
    wv_t = nc.dram_tensor("Wv", [D, D], f32, kind="ExternalInput")
    bq_t = nc.dram_tensor("bq", [D], f32, kind="ExternalInput")
    ww_t = nc.dram_tensor("Ww", [1, D], f32, kind="ExternalInput")
    nd_t = nc.dram_tensor("nd", [NT, 128], f32, kind="ExternalOutput")

    Exp = mybir.ActivationFunctionType.Exp

    with tile.TileContext(nc) as tc:
        with tc.tile_pool(name="singles", bufs=1) as singles:
            # Dense PE burst to flip the HAM clock gate to 8/8 (~3.4us of
            # sustained PE activity) while DMAs stream in; uses a memset
            # dummy so it does not wait on the gpsimd-built identity.
            dummy = singles.tile([128, 128], f32)
            nc.vector.memset(dummy, 1.0)
            with tc.tile_pool(name="ps_warm", bufs=1, space="PSUM") as ps_warm:
                warm_ps = ps_warm.tile([128, 128], f32, tag="warm")
                for _ in range(14):
                    nc.tensor.matmul(warm_ps, lhsT=dummy, rhs=dummy, start=True, stop=True)
            ident = singles.tile([128, 128], f32)
            make_identity(nc, ident)
            identb = singles.tile([128, 128], bf16)
            nc.vector.tensor_copy(identb, ident)

            # Weights natural layout: [e_within_chunk(128), e_chunk(2), col(256)]
            wq_sb = singles.tile([128, 2, D], f32)
            nc.sync.dma_start(out=wq_sb, in_=wq_t.ap().rearrange("(c p) d -> p c d", p=128))
            wk_sb = singles.tile([128, 2, D], f32)
            nc.sync.dma_start(out=wk_sb, in_=wk_t.ap().rearrange("(c p) d -> p c d", p=128))
            wv_sb = singles.tile([128, 2, D], f32)
            nc.sync.dma_start(out=wv_sb, in_=wv_t.ap().rearrange("(c p) d -> p c d", p=128))
            bq_sb = singles.tile([128, 2], f32)
            nc.sync.dma_start(out=bq_sb, in_=bq_t.ap().rearrange("(c p) -> p c", p=128))
            ww_sb = singles.tile([128, 2], f32)
            nc.sync.dma_start(out=ww_sb, in_=ww_t.ap().rearrange("o (c p) -> p (o c)", p=128))

            # x layout: partition p, column-block t holds token m = p*16 + t
            # (16KB contiguous per partition per DMA -> full DMA bandwidth).
            # This permutes the token order; softmax sums over tokens are
            # permutation-invariant and the host un-permutes the outputs.
            x_sb = singles.tile([128, NT, D], f32)
            x_dram = x_t.ap().rearrange("(p t) d -> p t d", p=128)
            for q in range(4):
                nc.sync.dma_start(out=x_sb[:, q * 4:(q + 1) * 4, :], in_=x_dram[:, q * 4:(q + 1) * 4, :])

            with tc.tile_pool(name="ps_set", bufs=1, space="PSUM") as ps_set, \
                 tc.tile_pool(name="ps_xp", bufs=3, space="PSUM") as ps_xp, \
                 tc.tile_pool(name="ps_q", bufs=1, space="PSUM") as ps_qp, \
                 tc.tile_pool(name="ps_fill", bufs=1, space="PSUM") as ps_fill:

                # PE filler: keeps the HAM activity monitor warm while the PE
                # would otherwise stall on DVE/DMA during the setup phase.
                fill_ps = ps_fill.tile([128, 512], f32, tag="fill")

                def pe_fill(k=1):
                    for _ in range(k):
                        nc.tensor.matmul(fill_ps[:, 0:128], lhsT=dummy, rhs=dummy,
                                         start=True, stop=True)

                # A[d, c] = sum_e Wq[e, d] Wk[e, c]  (then scaled by 1/sqrt(D))
                A_sb = singles.tile([128, 2, D], bf16)
                for dch in range(2):
                    a_ps = ps_set.tile([128, D], f32, tag="a_ps")
                    for ech in range(2):
                        nc.tensor.matmul(
                            a_ps,
                            lhsT=wq_sb[:, ech, dch * 128:(dch + 1) * 128],
                            rhs=wk_sb[:, ech, :],
                            start=(ech == 0), stop=(ech == 1),
                        )
                    nc.vector.tensor_scalar_mul(A_sb[:, dch, :], a_ps, SCALE)

                # g[c] = sum_e Wk[e, c] bq[e] (scaled); h[c] = sum_e Wv[e, c] Ww[0, e]
                # NOTE: each output column's accumulation pair must be
                # consecutive — start=True clears has_written for the WHOLE
                # bank, so interleaving two accumulation groups in one bank
                # corrupts the earlier one.
                misc_ps = ps_set.tile([128, 8], f32, tag="a_ps")
                for cch in range(2):
                    for ech in range(2):
                        nc.tensor.matmul(
                            misc_ps[:, cch:cch + 1],
                            lhsT=wk_sb[:, ech, cch * 128:(cch + 1) * 128],
                            rhs=bq_sb[:, ech:ech + 1],
                            start=(ech == 0), stop=(ech == 1),
                        )
                for cch in range(2):
                    for ech in range(2):
                        nc.tensor.matmul(
                            misc_ps[:, 2 + cch:3 + cch],
                            lhsT=wv_sb[:, ech, cch * 128:(cch + 1) * 128],
                            rhs=ww_sb[:, ech:ech + 1],
                            start=(ech == 0), stop=(ech == 1),
                        )
                g_sb = singles.tile([128, 2], f32)
                nc.vector.tensor_scalar_mul(g_sb, misc_ps[:, 0:2], SCALE)
                h_sb = singles.tile([128, 2], f32)
                nc.vector.tensor_copy(h_sb, misc_ps[:, 2:4])

                # xT[c, m] via PE transposes (4 blocks per PSUM bank).
                # Interleaved with the QT matmuls so the PE stays dense:
                # tg 0-1 -> QT half 0 -> tg 2-3 -> QT half 1.
                # x is converted to bf16 first so the transposes run at
                # 1 cycle/row (vs 2 for fp32).
                xbf_sb = singles.tile([128, NT, D], bf16)
                xT_sb = singles.tile([128, 2, N], bf16)
                qt_sb = singles.tile([128, 2, N], bf16)
                # h_mat[c, j] = h[c] for all j (bf16), so a single matmul
                # produces w broadcast across all output partitions:
                # w_bcast[p, m] = sum_c h_mat[c, p] xT[c, m] = w[m].
                hmat_sb = singles.tile([128, 2, 128], bf16)
                zero_sb = singles.tile([128, 128], f32)
                nc.vector.memset(zero_sb, 0.0)
                wb_sb = singles.tile([128, N], bf16)

                def convert_group(tg):
                    nc.vector.tensor_copy(xbf_sb[:, tg * 4:(tg + 1) * 4, :],
                                          x_sb[:, tg * 4:(tg + 1) * 4, :])

                def transpose_group(tg):
                    for dch in range(2):
                        xp_ps = ps_xp.tile([128, 512], bf16, tag="xp")
                        for i in range(4):
                            tch = tg * 4 + i
                            nc.tensor.transpose(
                                xp_ps[:, i * 128:(i + 1) * 128],
                                xbf_sb[:, tch, dch * 128:(dch + 1) * 128],
                                identb,
                            )
                        nc.vector.tensor_copy(xT_sb[:, dch, tg * 512:(tg + 1) * 512], xp_ps)

                def qt_half(nh):
                    # QT[c, n] = sum_d A[d, c] xT[d, n] + g[c]   (A, g pre-scaled)
                    for cch in range(2):
                        q_ps = ps_qp.tile([128, 1024], f32, tag="q")
                        for nb in range(2):
                            for dch in range(2):
                                nc.tensor.matmul(
                                    q_ps[:, nb * 512:(nb + 1) * 512],
                                    lhsT=A_sb[:, dch, cch * 128:(cch + 1) * 128],
                                    rhs=xT_sb[:, dch, nh * 1024 + nb * 512: nh * 1024 + (nb + 1) * 512],
                                    start=(dch == 0), stop=(dch == 1),
                                )
                        nc.vector.tensor_scalar_add(
                            qt_sb[:, cch, nh * 1024:(nh + 1) * 1024], q_ps, g_sb[:, cch:cch + 1]
                        )

                def w_bcast_cols(b0, b1):
                    # w_bcast[p, m] = sum_c h[c] xT[c, m], identical on every
                    # partition (h_mat columns are all h).
                    for blk in range(b0, b1):
                        wb_ps = ps_xp.tile([128, 512], f32, tag="xp")
                        for cch in range(2):
                            nc.tensor.matmul(
                                wb_ps,
                                lhsT=hmat_sb[:, cch, :],
                                rhs=xT_sb[:, cch, blk * 512:(blk + 1) * 512],
                                start=(cch == 0), stop=(cch == 1),
                            )
                        nc.vector.tensor_copy(wb_sb[:, blk * 512:(blk + 1) * 512], wb_ps)

                for cch in range(2):
                    nc.vector.tensor_scalar_add(hmat_sb[:, cch, :], zero_sb, h_sb[:, cch:cch + 1])
                convert_group(0)
                convert_group(1)
                transpose_group(0)
                transpose_group(1)
                pe_fill(2)
                qt_half(0)
                convert_group(2)
                convert_group(3)
                w_bcast_cols(0, 2)
                pe_fill(2)
                transpose_group(2)
                transpose_group(3)
                pe_fill(2)
                qt_half(1)
                w_bcast_cols(2, 4)
                pe_fill(2)

            # Main loop (S layout, n on partitions): one full-row S tile per
            # n-chunk -> exp on ACT (denominator via accum_out) -> numerator
            # via DVE scalar_tensor_tensor against the pre-broadcast w.
            with tc.tile_pool(name="e_pool", bufs=3) as e_pool, \
                 tc.tile_pool(name="scr_pool", bufs=3) as scr_pool, \
                 tc.tile_pool(name="fin_pool", bufs=1) as fin_pool:
                dn_sb = fin_pool.tile([128, NT], f32)
                nm_sb = fin_pool.tile([128, NT], f32)
                with tc.tile_pool(name="ps_s", bufs=2, space="PSUM") as ps_s:
                    for nq in range(NT):
                        s_ps = ps_s.tile([128, 2048], f32, tag="s")
                        for nb in range(4):
                            for cch in range(2):
                                nc.tensor.matmul(
                                    s_ps[:, nb * 512:(nb + 1) * 512],
                                    lhsT=qt_sb[:, cch, nq * 128:(nq + 1) * 128],
                                    rhs=xT_sb[:, cch, nb * 512:(nb + 1) * 512],
                                    start=(cch == 0), stop=(cch == 1),
                                )
                        e_sb = e_pool.tile([128, 2048], bf16, tag="e")
                        nc.scalar.activation(e_sb, s_ps, Exp,
                                             accum_out=dn_sb[:, nq:nq + 1])
                        scr = scr_pool.tile([128, 2048], bf16, tag="scr")
                        nc.vector.scalar_tensor_tensor(
                            out=scr,
                            in0=e_sb,
                            scalar=1.0,
                            in1=wb_sb,
                            op0=mybir.AluOpType.mult,
                            op1=mybir.AluOpType.mult,
                            accum_out=nm_sb[:, nq:nq + 1],
                        )
                # scores[p, nq] = numer/denom; output token c = nq*128 + p
                with tc.tile_pool(name="ps_fin", bufs=1, space="PSUM") as ps_fin:
                    rden = fin_pool.tile([128, NT], f32)
                    nc.vector.reciprocal(rden, dn_sb)
                    sc = fin_pool.tile([128, NT], f32)
                    nc.vector.tensor_mul(sc, nm_sb, rden)
                    sct_ps = ps_fin.tile([NT, 128], f32, tag="sct")
                    nc.tensor.transpose(sct_ps, sc, ident)
                    sct = fin_pool.tile([NT, 128], f32)
                    nc.vector.tensor_copy(sct, sct_ps)
                    nc.sync.dma_start(out=nd_t.ap(), in_=sct)

    nc.compile()
    return nc


def _get_nc():
    if "nc" not in _CACHE:
        _CACHE["nc"] = _build_nc()
    return _CACHE["nc"]


def run(inputs, trace=False, tmpdir=None):
    """Run on hardware. Returns (out [B, N] float32, exec_time_ns or None)."""
    from concourse.bass_utils import run_bass_kernel_spmd

    nc = _get_nc()
    x = np.ascontiguousarray(np.asarray(inputs["x"], dtype=np.float32))
    Wq = np.ascontiguousarray(np.asarray(inputs["Wq"], dtype=np.float32))
    Wk = np.ascontiguousarray(np.asarray(inputs["Wk"], dtype=np.float32))
    Wv = np.ascontiguousarray(np.asarray(inputs["Wv"], dtype=np.float32))
    bq = np.ascontiguousarray(np.asarray(inputs["bq"], dtype=np.float32))
    Ww = np.ascontiguousarray(np.asarray(inputs["Ww"], dtype=np.float32))
    bv = np.asarray(inputs["bv"], dtype=np.float32)
    bw = np.asarray(inputs["bw"], dtype=np.float32)

    in_maps = [
        {"x": np.ascontiguousarray(x[b]), "Wq": Wq, "Wk": Wk, "Wv": Wv, "bq": bq, "Ww": Ww}
        for b in range(B)
    ]
    res = run_bass_kernel_spmd(
        nc, in_maps, list(range(B)), trace=trace, tmpdir=tmpdir
    )

    # Host epilogue: add the constant (bv . Ww + bw) and un-permute tokens
    # (device token order: token m lives at flat position (m % 16) * 128
    # + m // 16 of the [NT, 128] output).
    c0bw = np.float32(bv @ Ww[0] + bw[0])
    m = np.arange(N)
    col_of_m = (m % 16) * 128 + m // 16
    out = np.empty((B, N), dtype=np.float32)
    for b in range(B):
        flat = res.results[b]["nd"].reshape(-1)
        out[b] = flat[col_of_m] + c0bw
    return out, res.exec_time_ns


def kernel(**inputs):
    out, _ = run(inputs, trace=False)
    return out

